# revision 1
# baseline (speedup 1.0000x reference)
"""Trainium2 Bass kernel for nn_DBGNN (gnn_message_passing).

Math (dead first-order branch eliminated; output depends only on):
    deg  = segment_sum([ew_ho, 1s], dst+self-loops)
    dinv = rsqrt(deg)
    agg  = segment_sum(x_h[src] * (dinv[src]*ew*dinv[dst]), dst)   # A_norm @ x_h
    xh   = elu(agg @ W_ho + b_ho)
    msg  = xh @ W_bip1 + b_bip1
    bip  = segment_sum(msg[bsrc], bdst, N)
    out  = elu(bip) @ W_lin + b_lin

Sharding: destination-node blocks of N/8 per core. Edges bucketed on host by
(core, 128-wide dst window, src half); per-window-group dma_gather of source
rows (int16 indices -> table split in two halves); one-hot-times-norm built
with one fused DVE tensor_scalar(is_equal, mult); aggregation as
PSUM-accumulated matmuls producing feature-major agg^T.  msg blocks are
AllGathered across cores; bipartite scatter + final linear per dst shard.

elu(x) = min(exp(x), max(x+1, 1)) - 1 exactly (exp(x) >= x+1 everywhere, and
for x<=0 exp(x) <= 1); the "-1" is folded into the next layer's bias.
"""
import sys

for _p in ("/opt/trn_rl_repo",):
    if _p not in sys.path:
        sys.path.append(_p)

import numpy as np

import concourse.bass as bass
import concourse.mybir as mybir
import concourse.tile as tile
from concourse import bacc
from concourse.bass_utils import run_bass_kernel_spmd

F32 = mybir.dt.float32
I16 = mybir.dt.int16

NCORES = 8
F = 128      # input/hidden feature dim
H1 = 64
C = 10
W = 128      # dst window width
WG = 1       # windows per dma_gather call


# ---------------------------------------------------------------------------
# host-side edge bucketing
# ---------------------------------------------------------------------------

def _wrap_idx(flat):
    """dma_gather index layout: unwrapped[i] = idx16[i % 16, i // 16],
    replicated to all 8 Q7 16-partition groups."""
    t16 = flat.reshape(-1, 16).T  # [16, len/16]
    return np.tile(t16, (8, 1)).astype(np.int16)


def _bucket_half(src, dst, wt, npc, nw, n_half):
    """Bucket one src-half's edges by (core, window). Returns M and per-core
    (idx_wrapped [128, nw*M*8] int16, dstloc [128, nw*M] f32,
    norm [128, nw*M] f32)."""
    core = dst // npc
    rem = dst - core * npc
    win = rem >> 7
    dstloc = (rem & 127).astype(np.float32)
    gwin = (core * nw + win).astype(np.int64)
    order = np.argsort(gwin, kind="stable")
    gwin_s = gwin[order]
    counts = np.bincount(gwin_s, minlength=NCORES * nw)
    M = max(1, int((counts.max() + 127) // 128))
    starts = np.zeros(NCORES * nw + 1, np.int64)
    np.cumsum(counts, out=starts[1:])
    src_s = src[order]
    dl_s = dstloc[order]
    w_s = wt[order]

    out = []
    for c in range(NCORES):
        gi = np.zeros((nw * M * 128,), np.int64)
        dl = np.zeros((nw * M * 128,), np.float32)
        nm = np.zeros((nw * M * 128,), np.float32)
        for w in range(nw):
            g = c * nw + w
            s0, s1 = starts[g], starts[g + 1]
            cnt = s1 - s0
            o = w * M * 128
            gi[o:o + cnt] = src_s[s0:s1]
            dl[o:o + cnt] = dl_s[s0:s1]
            nm[o:o + cnt] = w_s[s0:s1]
        assert gi.max(initial=0) < n_half <= 32768
        out.append((
            _wrap_idx(gi),
            np.ascontiguousarray(dl.reshape(nw * M, 128).T),
            np.ascontiguousarray(nm.reshape(nw * M, 128).T),
        ))
    return M, out


def _bucket_edges(src, dst, wt, half, npc, nw):
    lo = src < half
    m_lo, b_lo = _bucket_half(src[lo], dst[lo], wt[lo], npc, nw, half)
    m_hi, b_hi = _bucket_half(src[~lo] - half, dst[~lo], wt[~lo], npc, nw, half)
    return m_lo, b_lo, m_hi, b_hi


# ---------------------------------------------------------------------------
# Bass program
# ---------------------------------------------------------------------------

def build_nc(cfg):
    n, npc, nw = cfg["N"], cfg["NPC"], cfg["NW"]
    ma_lo, ma_hi = cfg["MA_LO"], cfg["MA_HI"]
    mb_lo, mb_hi = cfg["MB_LO"], cfg["MB_HI"]
    half_a = cfg["HALF_A"]
    msg_pad = nw * 128
    n_msg = NCORES * msg_pad
    half_b = cfg["HALF_B"]
    rep = cfg.get("REPEAT", 1)

    nc = bacc.Bacc("TRN2", target_bir_lowering=False, debug=False,
                   num_devices=NCORES)

    env = {}
    e = env

    e["xlo_t"] = nc.dram_tensor("x_lo", [half_a, F], F32, kind="ExternalInput")
    e["xhi_t"] = nc.dram_tensor("x_hi", [n - half_a, F], F32, kind="ExternalInput")
    for nm, m in (("alo", ma_lo), ("ahi", ma_hi)):
        e[nm + "i_t"] = nc.dram_tensor(nm + "_idx", [128, nw * m * 8], I16,
                                       kind="ExternalInput")
        e[nm + "d_t"] = nc.dram_tensor(nm + "_dst", [128, nw * m], F32,
                                       kind="ExternalInput")
        e[nm + "n_t"] = nc.dram_tensor(nm + "_nrm", [128, nw * m], F32,
                                       kind="ExternalInput")
    for nm, m in (("blo", mb_lo), ("bhi", mb_hi)):
        e[nm + "i_t"] = nc.dram_tensor(nm + "_idx", [128, nw * m * 8], I16,
                                       kind="ExternalInput")
        e[nm + "d_t"] = nc.dram_tensor(nm + "_dst", [128, nw * m], F32,
                                       kind="ExternalInput")
        e[nm + "n_t"] = nc.dram_tensor(nm + "_nrm", [128, nw * m], F32,
                                       kind="ExternalInput")
    e["iota_t"] = nc.dram_tensor("iota", [128, W], F32, kind="ExternalInput")
    e["who_t"] = nc.dram_tensor("w_ho", [F, F], F32, kind="ExternalInput")
    e["bho_t"] = nc.dram_tensor("b_ho", [F, 1], F32, kind="ExternalInput")
    e["wbip_t"] = nc.dram_tensor("w_bip", [F, H1], F32, kind="ExternalInput")
    e["bbip_t"] = nc.dram_tensor("b_bip", [1, H1], F32, kind="ExternalInput")
    e["wlin_t"] = nc.dram_tensor("w_lin", [H1, C], F32, kind="ExternalInput")
    e["blin_t"] = nc.dram_tensor("b_lin", [1, C], F32, kind="ExternalInput")
    e["out_t"] = nc.dram_tensor("outT", [C, npc], F32, kind="ExternalOutput")

    with tile.TileContext(nc) as tc:
        from contextlib import ExitStack
        with ExitStack() as ctx:
            const = ctx.enter_context(tc.tile_pool(name="const", bufs=1))
            meta = ctx.enter_context(tc.tile_pool(name="meta", bufs=1))
            work = ctx.enter_context(tc.tile_pool(name="work", bufs=1))

            sb = {}
            iota_sb = const.tile([128, W], F32)
            nc.sync.dma_start(out=iota_sb[:], in_=e["iota_t"].ap()[:, :])
            sb["iota"] = iota_sb
            for k, shape in (("who", [F, F]), ("bho", [F, 1]),
                             ("wbip", [F, H1]), ("bbip", [1, H1]),
                             ("wlin", [H1, C]), ("blin", [1, C])):
                t = const.tile(shape, F32, name=k + "_sb")
                nc.sync.dma_start(out=t[:], in_=e[k + "_t"].ap()[:, :])
                sb[k] = t
            ones_sb = const.tile([1, 512], F32)
            nc.vector.memset(ones_sb[:], 1.0)
            sb["ones"] = ones_sb
            bho1_sb = const.tile([F, 1], F32)
            nc.vector.tensor_scalar_add(out=bho1_sb[:], in0=sb["bho"][:],
                                        scalar1=1.0)
            sb["bho1"] = bho1_sb

            for nm, m in (("alo", ma_lo), ("ahi", ma_hi),
                          ("blo", mb_lo), ("bhi", mb_hi)):
                ti = meta.tile([128, nw * m * 8], I16, name=nm + "i_sb")
                nc.sync.dma_start(out=ti[:], in_=e[nm + "i_t"].ap()[:, :])
                td = meta.tile([128, nw * m], F32, name=nm + "d_sb")
                nc.sync.dma_start(out=td[:], in_=e[nm + "d_t"].ap()[:, :])
                tn = meta.tile([128, nw * m], F32, name=nm + "n_sb")
                nc.sync.dma_start(out=tn[:], in_=e[nm + "n_t"].ap()[:, :])
                sb[nm] = (ti, td, tn)

            e["cc_in"] = nc.dram_tensor("cc_in", [msg_pad, H1], F32,
                                        kind="Internal")
            e["cc_out"] = nc.dram_tensor("cc_out", [n_msg, H1], F32,
                                         kind="Internal", addr_space="Shared")
            # dma_gather from Shared address space hangs on HW; bounce the
            # all-gathered table into Local DRAM and gather from there.
            e["cc_loc"] = nc.dram_tensor("cc_loc", [n_msg, H1], F32,
                                         kind="Internal")

            for r in range(rep):
                _body(nc, tc, cfg, e, sb, work, r)

    nc.compile()
    return nc


def _body(nc, tc, cfg, e, sb, work, r):
    from contextlib import ExitStack
    n, npc, nw = cfg["N"], cfg["NPC"], cfg["NW"]
    ma_lo, ma_hi = cfg["MA_LO"], cfg["MA_HI"]
    mb_lo, mb_hi = cfg["MB_LO"], cfg["MB_HI"]
    half_b = cfg["HALF_B"]
    msg_pad = nw * 128

    iota_sb, ones_sb = sb["iota"], sb["ones"]
    who_sb, bho_sb, bho1_sb = sb["who"], sb["bho"], sb["bho1"]
    wbip_sb, bbip_sb = sb["wbip"], sb["bbip"]
    wlin_sb, blin_sb = sb["wlin"], sb["blin"]
    xlo_t, xhi_t, out_t = e["xlo_t"], e["xhi_t"], e["out_t"]
    cc_in, cc_out, cc_loc = e["cc_in"], e["cc_out"], e["cc_loc"]

    import os
    stage_lim = os.environ.get("GNN_STAGE", "full")
    aggT = work.tile([128, npc], F32, tag="aggT", name=f"aggT{r}")
    xhT = work.tile([128, npc], F32, tag="xhT", name=f"xhT{r}")
    bipT = work.tile([H1, nw * 128], F32, tag="bipT", name=f"bipT{r}")

    with ExitStack() as stk:
        gA = stk.enter_context(tc.tile_pool(name="gA", bufs=2))
        sA = stk.enter_context(tc.tile_pool(name="sA", bufs=6))
        psA = stk.enter_context(tc.tile_pool(name="psA", bufs=4, space="PSUM"))
        psB = stk.enter_context(tc.tile_pool(name="psB", bufs=2, space="PSUM"))
        psC = stk.enter_context(tc.tile_pool(name="psC", bufs=2, space="PSUM"))
        sB = stk.enter_context(tc.tile_pool(name="sB", bufs=3))

        # ============ stage A: agg^T[f, dst] = sum_e norm_e x_h[src_e, f] ====
        for g0 in range(0, nw, WG):
            wn = min(WG, nw - g0)
            Gs = {}
            for nm, m, tbl in (("alo", ma_lo, xlo_t), ("ahi", ma_hi, xhi_t)):
                idx_sb = sb[nm][0]
                Gt = gA.tile([128, WG * m, F], F32, tag="G" + nm,
                             name=f"G{nm}_{r}_{g0}")
                nc.gpsimd.dma_gather(
                    out_ap=Gt[:, :wn * m, :],
                    in_ap=tbl.ap()[:, :],
                    idxs_ap=idx_sb[:, g0 * m * 8:(g0 + wn) * m * 8],
                    num_idxs=wn * m * 128,
                    num_idxs_reg=wn * m * 128,
                    elem_size=F, single_packet=False)
                Gs[nm] = Gt
            for wi in range(wn):
                w = g0 + wi
                wlen = min(128, npc - w * 128)
                acc = psA.tile([128, W], F32, tag="accA", space="PSUM",
                               name=f"accA{r}_{w}")
                total = ma_lo + ma_hi
                tt = 0
                for nm, m in (("alo", ma_lo), ("ahi", ma_hi)):
                    _, dstb, nrmb = sb[nm]
                    Gt = Gs[nm]
                    for t in range(m):
                        col = w * m + t
                        S = sA.tile([128, W], F32, tag="S", name=f"S{r}_{w}_{nm}{t}")
                        nc.vector.tensor_scalar(
                            out=S[:], in0=iota_sb[:],
                            scalar1=dstb[:, col:col + 1],
                            scalar2=nrmb[:, col:col + 1],
                            op0=mybir.AluOpType.is_equal,
                            op1=mybir.AluOpType.mult)
                        nc.tensor.matmul(out=acc[:], lhsT=Gt[:, wi * m + t, :],
                                         rhs=S[:], start=(tt == 0),
                                         stop=(tt == total - 1))
                        tt += 1
                nc.scalar.copy(out=aggT[:, w * 128:w * 128 + wlen],
                               in_=acc[:, :wlen])

        # ============ stage B: xh' = min(exp(z), z+1), z = W_ho^T aggT + b ===
        for j in range((npc + 511) // 512):
            nt = min(512, npc - j * 512)
            zB = psB.tile([128, 512], F32, tag="zB", name=f"zB{r}_{j}",
                          space="PSUM")
            nc.tensor.matmul(out=zB[:, :nt], lhsT=who_sb[:],
                             rhs=aggT[:, j * 512:j * 512 + nt],
                             start=True, stop=True)
            eB = sB.tile([128, 512], F32, tag="eB", name=f"eB{r}_{j}")
            nc.scalar.activation(out=eB[:, :nt], in_=zB[:, :nt],
                                 func=mybir.ActivationFunctionType.Exp,
                                 bias=bho_sb[:], scale=1.0)
            zbB = sB.tile([128, 512], F32, tag="zbB", name=f"zbB{r}_{j}")
            nc.vector.tensor_scalar(out=zbB[:, :nt], in0=zB[:, :nt],
                                    scalar1=bho1_sb[:], scalar2=1.0,
                                    op0=mybir.AluOpType.add,
                                    op1=mybir.AluOpType.max)
            nc.vector.tensor_tensor(out=xhT[:, j * 512:j * 512 + nt],
                                    in0=eB[:, :nt], in1=zbB[:, :nt],
                                    op=mybir.AluOpType.min)

        if stage_lim == "A":
            for j in range((npc + 511) // 512):
                nt = min(512, npc - j * 512)
                oX = sB.tile([C, 512], F32, tag="oX", name=f"oX{r}_{j}")
                nc.vector.tensor_copy(out=oX[:, :nt], in_=xhT[:C, j * 512:j * 512 + nt])
                nc.sync.dma_start(out=out_t.ap()[:, j * 512:j * 512 + nt],
                                  in_=oX[:, :nt])
            return

        # ============ stage C: msg = xh' W_bip + b'  (node-major tiles) ======
        if npc % 128 != 0:
            # zero the padded tail rows of cc_in once (gathers may touch them)
            zpad = sB.tile([msg_pad - npc, H1], F32, tag="zpad",
                           name=f"zpad{r}")
            nc.vector.memset(zpad[:], 0.0)
            nc.sync.dma_start(out=cc_in.ap()[npc:, :], in_=zpad[:])
        for t in range(nw):
            ntl = min(128, npc - t * 128)
            zC = psC.tile([128, H1], F32, tag="zC", name=f"zC{r}_{t}",
                          space="PSUM")
            nc.tensor.matmul(out=zC[:ntl, :],
                             lhsT=xhT[:, t * 128:t * 128 + ntl],
                             rhs=wbip_sb[:], start=True, stop=False)
            nc.tensor.matmul(out=zC[:ntl, :], lhsT=ones_sb[:, :ntl],
                             rhs=bbip_sb[:], start=False, stop=True)
            oC = sB.tile([128, H1], F32, tag="oC", name=f"oC{r}_{t}")
            nc.vector.tensor_copy(out=oC[:ntl, :], in_=zC[:ntl, :])
            nc.sync.dma_start(out=cc_in.ap()[t * 128:t * 128 + ntl, :],
                              in_=oC[:ntl, :])

        nc.gpsimd.collective_compute(
            kind="AllGather", op=mybir.AluOpType.bypass,
            replica_groups=[list(range(NCORES))],
            ins=[cc_in.ap()[:, :]], outs=[cc_out.ap()[:, :]])
        nc.sync.dma_start(out=cc_loc.ap()[:, :], in_=cc_out.ap()[:, :])

        if stage_lim == "C":
            for j in range((npc + 511) // 512):
                nt = min(512, npc - j * 512)
                oY = sB.tile([C, 512], F32, tag="oY", name=f"oY{r}_{j}")
                nc.vector.tensor_copy(out=oY[:, :nt],
                                      in_=xhT[:C, j * 512:j * 512 + nt])
                nc.sync.dma_start(out=out_t.ap()[:, j * 512:j * 512 + nt],
                                  in_=oY[:, :nt])
            return

    # ============ stage D: bip' = exp-min of bipartite scatter ===============
    with ExitStack() as stk2:
        gD = stk2.enter_context(tc.tile_pool(name="gD", bufs=2))
        sD = stk2.enter_context(tc.tile_pool(name="sD", bufs=6))
        psD = stk2.enter_context(tc.tile_pool(name="psD", bufs=3, space="PSUM"))
        psF = stk2.enter_context(tc.tile_pool(name="psF", bufs=2, space="PSUM"))
        sF = stk2.enter_context(tc.tile_pool(name="sF", bufs=3))

        BG = 4  # windows per bipartite gather group
        for g0 in range(0, nw, BG):
            wn = min(BG, nw - g0)
            Gs = {}
            for nm, m, tbl in (("blo", mb_lo, cc_loc.ap()[:half_b, :]),
                               ("bhi", mb_hi, cc_loc.ap()[half_b:, :])):
                idx_sb = sb[nm][0]
                Gt = gD.tile([128, BG * m, H1], F32, tag="G" + nm,
                             name=f"G{nm}_{r}_{g0}")
                nc.gpsimd.dma_gather(
                    out_ap=Gt[:, :wn * m, :],
                    in_ap=tbl,
                    idxs_ap=idx_sb[:, g0 * m * 8:(g0 + wn) * m * 8],
                    num_idxs=wn * m * 128,
                    num_idxs_reg=wn * m * 128,
                    elem_size=H1, single_packet=False)
                Gs[nm] = Gt
            for wi in range(wn):
                w = g0 + wi
                accD = psD.tile([H1, W], F32, tag="accD", space="PSUM",
                                name=f"accD{r}_{w}")
                total = mb_lo + mb_hi
                tt = 0
                for nm, m in (("blo", mb_lo), ("bhi", mb_hi)):
                    _, dstb, nrmb = sb[nm]
                    Gt = Gs[nm]
                    for t in range(m):
                        col = w * m + t
                        Sb = sD.tile([128, W], F32, tag="Sb",
                                     name=f"Sb{r}_{w}_{nm}{t}")
                        nc.vector.tensor_scalar(
                            out=Sb[:], in0=iota_sb[:],
                            scalar1=dstb[:, col:col + 1],
                            scalar2=nrmb[:, col:col + 1],
                            op0=mybir.AluOpType.is_equal,
                            op1=mybir.AluOpType.mult)
                        nc.tensor.matmul(out=accD[:],
                                         lhsT=Gt[:, wi * m + t, :],
                                         rhs=Sb[:], start=(tt == 0),
                                         stop=(tt == total - 1))
                        tt += 1
                eD = sD.tile([H1, W], F32, tag="eD", name=f"eD{r}_{w}")
                nc.scalar.activation(out=eD[:], in_=accD[:],
                                     func=mybir.ActivationFunctionType.Exp)
                zbD = sD.tile([H1, W], F32, tag="zbD", name=f"zbD{r}_{w}")
                nc.vector.tensor_scalar(out=zbD[:], in0=accD[:],
                                        scalar1=1.0, scalar2=1.0,
                                        op0=mybir.AluOpType.add,
                                        op1=mybir.AluOpType.max)
                nc.vector.tensor_tensor(out=bipT[:, w * 128:(w + 1) * 128],
                                        in0=eD[:], in1=zbD[:],
                                        op=mybir.AluOpType.min)

        # ============ stage F: out^T = W_lin'^T bip' + b'' ===================
        for j in range((npc + 511) // 512):
            nt = min(512, npc - j * 512)
            zF = psF.tile([C, 512], F32, tag="zF", name=f"zF{r}_{j}",
                          space="PSUM")
            nc.tensor.matmul(out=zF[:, :nt], lhsT=wlin_sb[:],
                             rhs=bipT[:, j * 512:j * 512 + nt],
                             start=True, stop=False)
            nc.tensor.matmul(out=zF[:, :nt], lhsT=blin_sb[:],
                             rhs=ones_sb[:, :nt], start=False, stop=True)
            oF = sF.tile([C, 512], F32, tag="oF", name=f"oF{r}_{j}")
            nc.vector.tensor_copy(out=oF[:, :nt], in_=zF[:, :nt])
            nc.sync.dma_start(out=out_t.ap()[:, j * 512:j * 512 + nt],
                              in_=oF[:, :nt])


# ---------------------------------------------------------------------------
# public entry
# ---------------------------------------------------------------------------

def _prepare(inputs, n):
    npc = n // NCORES
    nw = (npc + 127) // 128
    half_a = (n + 1) // 2

    ei = np.asarray(inputs["edge_index_higher_order"])
    src = ei[0].astype(np.int64)
    dst = ei[1].astype(np.int64)
    ew = np.asarray(inputs["edge_weights_higher_order"]).astype(np.float64)

    deg = np.bincount(dst, weights=ew, minlength=n) + 1.0
    dinv = 1.0 / np.sqrt(deg)
    norm = (dinv[src] * ew * dinv[dst]).astype(np.float32)

    loops = np.arange(n, dtype=np.int64)
    src_all = np.concatenate([src, loops])
    dst_all = np.concatenate([dst, loops])
    nrm_all = np.concatenate([norm, (dinv * dinv).astype(np.float32)])

    ma_lo, blo_a, ma_hi, bhi_a = _bucket_edges(src_all, dst_all, nrm_all,
                                               half_a, npc, nw)

    bi = np.asarray(inputs["bipartite_edge_index"])
    bsrc = bi[0].astype(np.int64)
    bdst = bi[1].astype(np.int64)
    msg_pad = nw * 128
    n_msg = NCORES * msg_pad
    half_b = n_msg // 2
    bsrc_remap = (bsrc // npc) * msg_pad + (bsrc % npc)
    mb_lo, blo_b, mb_hi, bhi_b = _bucket_edges(
        bsrc_remap, bdst, np.ones(len(bsrc), np.float32), half_b, npc, nw)

    cfg = dict(N=n, NPC=npc, NW=nw, HALF_A=half_a, HALF_B=half_b,
               MA_LO=ma_lo, MA_HI=ma_hi, MB_LO=mb_lo, MB_HI=mb_hi)
    buckets = dict(alo=blo_a, ahi=bhi_a, blo=blo_b, bhi=bhi_b)
    return cfg, buckets


def make_in_maps(inputs, cfg, buckets):
    n = cfg["N"]
    half_a = cfg["HALF_A"]
    x_h = np.ascontiguousarray(np.asarray(inputs["x_h"], dtype=np.float32))

    W_ho = np.asarray(inputs["W_ho"], np.float32)
    b_ho = np.asarray(inputs["b_ho"], np.float32)
    W_bip = np.asarray(inputs["W_bip1"], np.float32)
    b_bip = np.asarray(inputs["b_bip1"], np.float32)
    W_lin = np.asarray(inputs["W_lin"], np.float32)
    b_lin = np.asarray(inputs["b_lin"], np.float32)

    b_bip_eff = (b_bip - W_bip.sum(axis=0)).reshape(1, H1).astype(np.float32)
    b_lin_eff = (b_lin - W_lin.sum(axis=0)).reshape(1, C).astype(np.float32)
    iota = np.broadcast_to(np.arange(W, dtype=np.float32), (128, W)).copy()

    in_maps = []
    for c in range(NCORES):
        m = {
            "x_lo": x_h[:half_a],
            "x_hi": x_h[half_a:],
            "iota": iota,
            "w_ho": np.ascontiguousarray(W_ho),
            "b_ho": b_ho.reshape(F, 1).astype(np.float32),
            "w_bip": np.ascontiguousarray(W_bip),
            "b_bip": b_bip_eff,
            "w_lin": np.ascontiguousarray(W_lin),
            "b_lin": b_lin_eff,
        }
        for nm in ("alo", "ahi", "blo", "bhi"):
            gi, dl, nr = buckets[nm][c]
            m[nm + "_idx"] = gi
            m[nm + "_dst"] = dl
            m[nm + "_nrm"] = nr
        in_maps.append(m)
    return in_maps


def kernel(**inputs):
    x_h = np.asarray(inputs["x_h"])
    n = x_h.shape[0]
    cfg, buckets = _prepare(inputs, n)
    nc = build_nc(cfg)
    in_maps = make_in_maps(inputs, cfg, buckets)
    res = run_bass_kernel_spmd(nc, in_maps, core_ids=list(range(NCORES)))
    npc = cfg["NPC"]
    out = np.empty((n, C), np.float32)
    for c in range(NCORES):
        out[c * npc:(c + 1) * npc] = res.results[c]["outT"].T
    return out



# revision 7
# speedup vs baseline: 1.1879x; 1.1879x over previous
"""Trainium2 Bass kernel for nn_DBGNN (gnn_message_passing), v2.

Math (dead first-order branch eliminated; output depends only on):
    deg  = segment_sum([ew_ho, 1s], dst+self-loops)
    dinv = rsqrt(deg)
    agg  = segment_sum(x_h[src] * (dinv[src]*ew*dinv[dst]), dst)   # A_norm @ x_h
    xh   = elu(agg @ W_ho + b_ho)
    msg  = xh @ W_bip1 + b_bip1
    bip  = segment_sum(msg[bsrc], bdst, N)
    out  = elu(bip) @ W_lin + b_lin

Sharding v2 — collective-free: core c owns output rows [c*npc, (c+1)*npc).
Its bipartite edges reference a set U_c of ~5.9k unique HO nodes; core c
recomputes the HO branch for exactly U_c (~17% duplicated work across cores)
so the bipartite gather-scatter is purely local.  No AllGather.

Compute layout per core:
  A: aggT[f, u]  = edge-tile matmuls: lhsT = dma_gathered x_h rows (bf16),
     rhs = one-hot(dstloc)*norm built by one DVE tensor_scalar per tile
     (bf16 in/out -> 4x_2p DVE mode), PSUM accumulated per 128-dst window.
  B: xhT = elu'(W_ho^T aggT + b) via exp/max/min trick (exact elu + 1).
  C: msg tiles = xhT_tile^T W_bip + b' -> bf16 rows of msg_dram[NU, 128]
     (64 real cols; gather stride must be 256B-divisible).
  D: bip^T = PSUM matmuls of gathered msg rows vs one-hot dst, then elu'.
  F: out^T = W_lin'^T bip' + b''.

elu(x) = min(exp(x), max(x+1, 1)) - 1 exactly; the "-1" is folded into the
next layer's bias (b_eff = b - W.sum(0)).
"""
import sys

for _p in ("/opt/trn_rl_repo",):
    if _p not in sys.path:
        sys.path.append(_p)

import numpy as np

import concourse.bass as bass
import concourse.mybir as mybir
import concourse.tile as tile
from concourse import bacc
from concourse.bass_utils import run_bass_kernel_spmd

F32 = mybir.dt.float32
BF16 = mybir.dt.bfloat16
I16 = mybir.dt.int16

NCORES = 8
F = 128      # input/hidden feature dim
H1 = 64
C = 10
WG_A = 4     # stage-A windows per dma_gather call
WG_D = 4     # stage-D windows per dma_gather call


# ---------------------------------------------------------------------------
# host-side edge bucketing
# ---------------------------------------------------------------------------

def _wrap_idx(flat):
    """dma_gather index layout: unwrapped[i] = idx16[i % 16, i // 16],
    replicated to all 8 Q7 16-partition groups."""
    t16 = flat.reshape(-1, 16).T  # [16, len/16]
    return np.tile(t16, (8, 1)).astype(np.int16)


def _bucket(src, u, wt, nw):
    """Bucket edges by 128-wide dst window of local index u.
    Returns per-window counts and sorted (src, dstloc, wt)."""
    win = u >> 7
    dstloc = (u & 127).astype(np.float32)
    order = np.argsort(win, kind="stable")
    win_s = win[order]
    counts = np.bincount(win_s, minlength=nw)
    return counts, src[order], dstloc[order], wt[order]


def _pack(counts, src_s, dl_s, w_s, nw, m):
    """Pack sorted edges into [nw*m*128] padded slots."""
    total = nw * m * 128
    gi = np.zeros((total,), np.int64)
    dl = np.zeros((total,), np.float32)
    nm = np.zeros((total,), np.float32)
    starts = np.zeros(nw + 1, np.int64)
    np.cumsum(counts, out=starts[1:])
    for w in range(nw):
        s0, s1 = starts[w], starts[w + 1]
        cnt = s1 - s0
        o = w * m * 128
        gi[o:o + cnt] = src_s[s0:s1]
        dl[o:o + cnt] = dl_s[s0:s1]
        nm[o:o + cnt] = w_s[s0:s1]
    return (
        _wrap_idx(gi),
        np.ascontiguousarray(dl.reshape(nw * m, 128).T),
        np.ascontiguousarray(nm.reshape(nw * m, 128).T),
    )


def _prepare(inputs, n):
    npc = n // NCORES
    nw = (npc + 127) // 128          # dst windows for stages D/F
    half_a = (n + 1) // 2

    ei = np.asarray(inputs["edge_index_higher_order"])
    src = ei[0].astype(np.int64)
    dst = ei[1].astype(np.int64)
    ew = np.asarray(inputs["edge_weights_higher_order"]).astype(np.float64)

    deg = np.bincount(dst, weights=ew, minlength=n) + 1.0
    dinv = 1.0 / np.sqrt(deg)
    norm = (dinv[src] * ew * dinv[dst]).astype(np.float32)

    loops = np.arange(n, dtype=np.int64)
    src_all = np.concatenate([src, loops])
    dst_all = np.concatenate([dst, loops])
    nrm_all = np.concatenate([norm, (dinv * dinv).astype(np.float32)])

    bi = np.asarray(inputs["bipartite_edge_index"])
    bsrc = bi[0].astype(np.int64)
    bdst = bi[1].astype(np.int64)
    bcore = bdst // npc

    # per-core unique HO source sets
    U = [np.unique(bsrc[bcore == c]) for c in range(NCORES)]
    nU = max(len(u) for u in U)
    nU_pad = ((nU + 127) // 128) * 128
    nwu = nU_pad // 128              # u-windows for stages A/B/C

    # ---- stage A buckets (HO edges targeting U_c, dst remapped to u) ----
    a_counts, a_sorted = [], []
    for c in range(NCORES):
        lut = np.full(n, -1, np.int64)
        lut[U[c]] = np.arange(len(U[c]))
        u_of = lut[dst_all]
        mask = u_of >= 0
        s_c, u_c, w_c = src_all[mask], u_of[mask], nrm_all[mask]
        lo = s_c < half_a
        ch, cs = [], []
        for msk, base in ((lo, 0), (~lo, half_a)):
            cnts, ss, dd, ww = _bucket(s_c[msk] - base, u_c[msk], w_c[msk], nwu)
            ch.append(cnts)
            cs.append((ss, dd, ww))
        a_counts.append(ch)
        a_sorted.append(cs)
    ma_lo = max(1, int((max(ch[0].max() for ch in a_counts) + 127) // 128))
    ma_hi = max(1, int((max(ch[1].max() for ch in a_counts) + 127) // 128))

    # ---- stage D buckets (bipartite edges, src remapped to u) ----
    b_counts, b_sorted = [], []
    for c in range(NCORES):
        lut = np.full(n, -1, np.int64)
        lut[U[c]] = np.arange(len(U[c]))
        sel = bcore == c
        u16 = lut[bsrc[sel]]
        assert u16.min(initial=0) >= 0
        dl = bdst[sel] - c * npc
        cnts, ss, dd, ww = _bucket(u16, dl, np.ones(len(u16), np.float32), nw)
        b_counts.append(cnts)
        b_sorted.append((ss, dd, ww))
    mb = max(1, int((max(bc.max() for bc in b_counts) + 127) // 128))

    buckets = dict(alo=[], ahi=[], b=[])
    for c in range(NCORES):
        buckets["alo"].append(
            _pack(a_counts[c][0], *a_sorted[c][0], nwu, ma_lo))
        buckets["ahi"].append(
            _pack(a_counts[c][1], *a_sorted[c][1], nwu, ma_hi))
        buckets["b"].append(_pack(b_counts[c], *b_sorted[c], nw, mb))

    cfg = dict(N=n, NPC=npc, NW=nw, NU=nU_pad, NWU=nwu, HALF_A=half_a,
               MA_LO=ma_lo, MA_HI=ma_hi, MB=mb)
    return cfg, buckets


# ---------------------------------------------------------------------------
# Bass program
# ---------------------------------------------------------------------------

def build_nc(cfg):
    n, npc, nw = cfg["N"], cfg["NPC"], cfg["NW"]
    nu, nwu = cfg["NU"], cfg["NWU"]
    ma_lo, ma_hi, mb = cfg["MA_LO"], cfg["MA_HI"], cfg["MB"]
    half_a = cfg["HALF_A"]
    rep = cfg.get("REPEAT", 1)

    nc = bacc.Bacc("TRN2", target_bir_lowering=False, debug=False,
                   num_devices=NCORES)

    e = {}
    e["xlo_t"] = nc.dram_tensor("x_lo", [half_a, F], BF16, kind="ExternalInput")
    e["xhi_t"] = nc.dram_tensor("x_hi", [n - half_a, F], BF16,
                                kind="ExternalInput")
    for nm, nwx, m in (("alo", nwu, ma_lo), ("ahi", nwu, ma_hi),
                       ("b", nw, mb)):
        e[nm + "i_t"] = nc.dram_tensor(nm + "_idx", [128, nwx * m * 8], I16,
                                       kind="ExternalInput")
        e[nm + "d_t"] = nc.dram_tensor(nm + "_dst", [128, nwx * m], F32,
                                       kind="ExternalInput")
        e[nm + "n_t"] = nc.dram_tensor(nm + "_nrm", [128, nwx * m], F32,
                                       kind="ExternalInput")
    e["iota_t"] = nc.dram_tensor("iota", [128, 128], BF16, kind="ExternalInput")
    e["who_t"] = nc.dram_tensor("w_ho", [F, F], F32, kind="ExternalInput")
    e["bho_t"] = nc.dram_tensor("b_ho", [F, 1], F32, kind="ExternalInput")
    e["wbip_t"] = nc.dram_tensor("w_bip", [F, H1], F32, kind="ExternalInput")
    e["bbip_t"] = nc.dram_tensor("b_bip", [1, H1], F32, kind="ExternalInput")
    e["wlin_t"] = nc.dram_tensor("w_lin", [H1, C], F32, kind="ExternalInput")
    e["blin_t"] = nc.dram_tensor("b_lin", [1, C], F32, kind="ExternalInput")
    e["out_t"] = nc.dram_tensor("outT", [C, npc], F32, kind="ExternalOutput")
    e["msg_t"] = nc.dram_tensor("msg", [nu, F], BF16, kind="Internal")

    with tile.TileContext(nc) as tc:
        from contextlib import ExitStack
        with ExitStack() as ctx:
            const = ctx.enter_context(tc.tile_pool(name="const", bufs=1))
            meta = ctx.enter_context(tc.tile_pool(name="meta", bufs=1))
            work = ctx.enter_context(tc.tile_pool(name="work", bufs=1))

            sb = {}
            iota_sb = const.tile([128, 128], BF16)
            nc.sync.dma_start(out=iota_sb[:], in_=e["iota_t"].ap()[:, :])
            sb["iota"] = iota_sb
            for k, shape, dt in (("who", [F, F], F32), ("bho", [F, 1], F32),
                                 ("wbip", [F, H1], F32),
                                 ("bbip", [1, H1], F32),
                                 ("wlin", [H1, C], F32),
                                 ("blin", [1, C], F32)):
                t = const.tile(shape, dt, name=k + "_sb")
                nc.sync.dma_start(out=t[:], in_=e[k + "_t"].ap()[:, :])
                sb[k] = t
            ones_sb = const.tile([1, 512], F32)
            nc.vector.memset(ones_sb[:], 1.0)
            sb["ones"] = ones_sb
            bho1_sb = const.tile([F, 1], F32)
            nc.vector.tensor_scalar_add(out=bho1_sb[:], in0=sb["bho"][:],
                                        scalar1=1.0)
            sb["bho1"] = bho1_sb

            for nm, nwx, m in (("alo", nwu, ma_lo), ("ahi", nwu, ma_hi),
                               ("b", nw, mb)):
                ti = meta.tile([128, nwx * m * 8], I16, name=nm + "i_sb")
                nc.sync.dma_start(out=ti[:], in_=e[nm + "i_t"].ap()[:, :])
                td = meta.tile([128, nwx * m], F32, name=nm + "d_sb")
                nc.sync.dma_start(out=td[:], in_=e[nm + "d_t"].ap()[:, :])
                tn = meta.tile([128, nwx * m], F32, name=nm + "n_sb")
                nc.sync.dma_start(out=tn[:], in_=e[nm + "n_t"].ap()[:, :])
                sb[nm] = (ti, td, tn)

            for r in range(rep):
                _body(nc, tc, cfg, e, sb, work, r)

    nc.compile()
    return nc


def _body(nc, tc, cfg, e, sb, work, r):
    from contextlib import ExitStack
    import os
    npc, nw = cfg["NPC"], cfg["NW"]
    nu, nwu = cfg["NU"], cfg["NWU"]
    ma_lo, ma_hi, mb = cfg["MA_LO"], cfg["MA_HI"], cfg["MB"]
    stage_lim = os.environ.get("GNN_STAGE", "full")

    iota_sb, ones_sb = sb["iota"], sb["ones"]
    who_sb, bho_sb, bho1_sb = sb["who"], sb["bho"], sb["bho1"]
    wbip_sb, bbip_sb = sb["wbip"], sb["bbip"]
    wlin_sb, blin_sb = sb["wlin"], sb["blin"]
    xlo_t, xhi_t, out_t, msg_t = e["xlo_t"], e["xhi_t"], e["out_t"], e["msg_t"]

    aggT = work.tile([128, nu], F32, tag="aggT", name=f"aggT{r}")
    xhT = work.tile([128, nu], F32, tag="xhT", name=f"xhT{r}")
    bipT = work.tile([H1, nw * 128], F32, tag="bipT", name=f"bipT{r}")

    with ExitStack() as stk:
        gA = stk.enter_context(tc.tile_pool(name="gA", bufs=2))
        sA = stk.enter_context(tc.tile_pool(name="sA", bufs=6))
        psA = stk.enter_context(tc.tile_pool(name="psA", bufs=4, space="PSUM"))
        psB = stk.enter_context(tc.tile_pool(name="psB", bufs=2, space="PSUM"))
        psC = stk.enter_context(tc.tile_pool(name="psC", bufs=2, space="PSUM"))
        sB = stk.enter_context(tc.tile_pool(name="sB", bufs=3))

        # ============ stage A: agg^T[f, u] = sum_e norm_e x_h[src_e, f] ====
        for g0 in range(0, nwu, WG_A):
            wn = min(WG_A, nwu - g0)
            Gs = {}
            for nm, m, tbl in (("alo", ma_lo, xlo_t), ("ahi", ma_hi, xhi_t)):
                idx_sb = sb[nm][0]
                Gt = gA.tile([128, WG_A * m, F], BF16, tag="G" + nm,
                             name=f"G{nm}_{r}_{g0}")
                nc.gpsimd.dma_gather(
                    out_ap=Gt[:, :wn * m, :],
                    in_ap=tbl.ap()[:, :],
                    idxs_ap=idx_sb[:, g0 * m * 8:(g0 + wn) * m * 8],
                    num_idxs=wn * m * 128,
                    num_idxs_reg=wn * m * 128,
                    elem_size=F, single_packet=False)
                Gs[nm] = Gt
            for wi in range(wn):
                w = g0 + wi
                acc = psA.tile([128, 128], F32, tag="accA", space="PSUM",
                               name=f"accA{r}_{w}")
                total = ma_lo + ma_hi
                tt = 0
                for nm, m in (("alo", ma_lo), ("ahi", ma_hi)):
                    _, dstb, nrmb = sb[nm]
                    Gt = Gs[nm]
                    for t in range(m):
                        col = w * m + t
                        S = sA.tile([128, 128], BF16, tag="S",
                                    name=f"S{r}_{w}_{nm}{t}")
                        nc.vector.tensor_scalar(
                            out=S[:], in0=iota_sb[:],
                            scalar1=dstb[:, col:col + 1],
                            scalar2=nrmb[:, col:col + 1],
                            op0=mybir.AluOpType.is_equal,
                            op1=mybir.AluOpType.mult)
                        nc.tensor.matmul(out=acc[:], lhsT=Gt[:, wi * m + t, :],
                                         rhs=S[:], start=(tt == 0),
                                         stop=(tt == total - 1))
                        tt += 1
                nc.scalar.copy(out=aggT[:, w * 128:(w + 1) * 128], in_=acc[:])

        # ============ stage B: xh' = min(exp(z), z+1), z = W_ho^T aggT + b ===
        for j in range((nu + 511) // 512):
            nt = min(512, nu - j * 512)
            zB = psB.tile([128, 512], F32, tag="zB", name=f"zB{r}_{j}",
                          space="PSUM")
            nc.tensor.matmul(out=zB[:, :nt], lhsT=who_sb[:],
                             rhs=aggT[:, j * 512:j * 512 + nt],
                             start=True, stop=True)
            eB = sB.tile([128, 512], F32, tag="eB", name=f"eB{r}_{j}")
            nc.scalar.activation(out=eB[:, :nt], in_=zB[:, :nt],
                                 func=mybir.ActivationFunctionType.Exp,
                                 bias=bho_sb[:], scale=1.0)
            zbB = sB.tile([128, 512], F32, tag="zbB", name=f"zbB{r}_{j}")
            nc.vector.tensor_scalar(out=zbB[:, :nt], in0=zB[:, :nt],
                                    scalar1=bho1_sb[:], scalar2=1.0,
                                    op0=mybir.AluOpType.add,
                                    op1=mybir.AluOpType.max)
            nc.vector.tensor_tensor(out=xhT[:, j * 512:j * 512 + nt],
                                    in0=eB[:, :nt], in1=zbB[:, :nt],
                                    op=mybir.AluOpType.min)

        if stage_lim == "A":
            dbg = min(nu, npc)
            for j in range((dbg + 511) // 512):
                nt = min(512, dbg - j * 512)
                oX = sB.tile([C, 512], F32, tag="oX", name=f"oX{r}_{j}")
                nc.vector.tensor_copy(out=oX[:, :nt],
                                      in_=xhT[:C, j * 512:j * 512 + nt])
                nc.sync.dma_start(out=out_t.ap()[:, j * 512:j * 512 + nt],
                                  in_=oX[:, :nt])
            return

        # ============ stage C: msg rows = xh'_tile W_bip + b'  (bf16) ======
        for t in range(nwu):
            zC = psC.tile([128, H1], F32, tag="zC", name=f"zC{r}_{t}",
                          space="PSUM")
            nc.tensor.matmul(out=zC[:], lhsT=xhT[:, t * 128:(t + 1) * 128],
                             rhs=wbip_sb[:], start=True, stop=False)
            nc.tensor.matmul(out=zC[:], lhsT=ones_sb[:, :128],
                             rhs=bbip_sb[:], start=False, stop=True)
            oC = sB.tile([128, H1], BF16, tag="oC", name=f"oC{r}_{t}")
            nc.vector.tensor_copy(out=oC[:], in_=zC[:])
            nc.scalar.dma_start(out=msg_t.ap()[t * 128:(t + 1) * 128, :H1],
                                in_=oC[:])

        if stage_lim == "C":
            dbg = min(nu, npc)
            for j in range((dbg + 511) // 512):
                nt = min(512, dbg - j * 512)
                oY = sB.tile([C, 512], F32, tag="oY", name=f"oY{r}_{j}")
                nc.vector.tensor_copy(out=oY[:, :nt],
                                      in_=xhT[:C, j * 512:j * 512 + nt])
                nc.sync.dma_start(out=out_t.ap()[:, j * 512:j * 512 + nt],
                                  in_=oY[:, :nt])
            return

    # ============ stage D: bip' = exp-min of bipartite scatter ===============
    with ExitStack() as stk2:
        gD = stk2.enter_context(tc.tile_pool(name="gD", bufs=2))
        sD = stk2.enter_context(tc.tile_pool(name="sD", bufs=6))
        psD = stk2.enter_context(tc.tile_pool(name="psD", bufs=3, space="PSUM"))
        psF = stk2.enter_context(tc.tile_pool(name="psF", bufs=2, space="PSUM"))
        sF = stk2.enter_context(tc.tile_pool(name="sF", bufs=3))

        bi_sb, bd_sb, bn_sb = sb["b"]
        for g0 in range(0, nw, WG_D):
            wn = min(WG_D, nw - g0)
            Gt = gD.tile([128, WG_D * mb, F], BF16, tag="Gb",
                         name=f"Gb_{r}_{g0}")
            nc.gpsimd.dma_gather(
                out_ap=Gt[:, :wn * mb, :],
                in_ap=msg_t.ap()[:, :],
                idxs_ap=bi_sb[:, g0 * mb * 8:(g0 + wn) * mb * 8],
                num_idxs=wn * mb * 128,
                num_idxs_reg=wn * mb * 128,
                elem_size=F, single_packet=False)
            for wi in range(wn):
                w = g0 + wi
                accD = psD.tile([H1, 128], F32, tag="accD", space="PSUM",
                                name=f"accD{r}_{w}")
                for t in range(mb):
                    col = w * mb + t
                    Sb = sD.tile([128, 128], BF16, tag="Sb",
                                 name=f"Sb{r}_{w}_{t}")
                    nc.vector.tensor_scalar(
                        out=Sb[:], in0=iota_sb[:],
                        scalar1=bd_sb[:, col:col + 1],
                        scalar2=bn_sb[:, col:col + 1],
                        op0=mybir.AluOpType.is_equal,
                        op1=mybir.AluOpType.mult)
                    nc.tensor.matmul(out=accD[:],
                                     lhsT=Gt[:, wi * mb + t, :H1],
                                     rhs=Sb[:], start=(t == 0),
                                     stop=(t == mb - 1))
                eD = sD.tile([H1, 128], F32, tag="eD", name=f"eD{r}_{w}")
                nc.scalar.activation(out=eD[:], in_=accD[:],
                                     func=mybir.ActivationFunctionType.Exp)
                zbD = sD.tile([H1, 128], F32, tag="zbD", name=f"zbD{r}_{w}")
                nc.vector.tensor_scalar(out=zbD[:], in0=accD[:],
                                        scalar1=1.0, scalar2=1.0,
                                        op0=mybir.AluOpType.add,
                                        op1=mybir.AluOpType.max)
                nc.vector.tensor_tensor(out=bipT[:, w * 128:(w + 1) * 128],
                                        in0=eD[:], in1=zbD[:],
                                        op=mybir.AluOpType.min)

        # ============ stage F: out^T = W_lin'^T bip' + b'' ===================
        for j in range((npc + 511) // 512):
            nt = min(512, npc - j * 512)
            zF = psF.tile([C, 512], F32, tag="zF", name=f"zF{r}_{j}",
                          space="PSUM")
            nc.tensor.matmul(out=zF[:, :nt], lhsT=wlin_sb[:],
                             rhs=bipT[:, j * 512:j * 512 + nt],
                             start=True, stop=False)
            nc.tensor.matmul(out=zF[:, :nt], lhsT=blin_sb[:],
                             rhs=ones_sb[:, :nt], start=False, stop=True)
            oF = sF.tile([C, 512], F32, tag="oF", name=f"oF{r}_{j}")
            nc.vector.tensor_copy(out=oF[:, :nt], in_=zF[:, :nt])
            nc.sync.dma_start(out=out_t.ap()[:, j * 512:j * 512 + nt],
                              in_=oF[:, :nt])


# ---------------------------------------------------------------------------
# public entry
# ---------------------------------------------------------------------------

def make_in_maps(inputs, cfg, buckets):
    half_a = cfg["HALF_A"]
    x_h = np.asarray(inputs["x_h"], dtype=np.float32)

    import ml_dtypes

    def bf(a):
        return np.ascontiguousarray(np.asarray(a).astype(ml_dtypes.bfloat16))

    W_ho = np.asarray(inputs["W_ho"], np.float32)
    b_ho = np.asarray(inputs["b_ho"], np.float32)
    W_bip = np.asarray(inputs["W_bip1"], np.float32)
    b_bip = np.asarray(inputs["b_bip1"], np.float32)
    W_lin = np.asarray(inputs["W_lin"], np.float32)
    b_lin = np.asarray(inputs["b_lin"], np.float32)

    b_bip_eff = (b_bip - W_bip.sum(axis=0)).reshape(1, H1)
    b_lin_eff = (b_lin - W_lin.sum(axis=0)).reshape(1, C)
    iota = np.broadcast_to(np.arange(128, dtype=np.float32), (128, 128))

    xlo16 = bf(x_h[:half_a])
    xhi16 = bf(x_h[half_a:])
    in_maps = []
    for c in range(NCORES):
        m = {
            "x_lo": xlo16,
            "x_hi": xhi16,
            "iota": bf(iota),
            "w_ho": np.ascontiguousarray(W_ho),
            "b_ho": b_ho.reshape(F, 1).astype(np.float32),
            "w_bip": np.ascontiguousarray(W_bip),
            "b_bip": b_bip_eff.astype(np.float32),
            "w_lin": np.ascontiguousarray(W_lin),
            "b_lin": b_lin_eff.astype(np.float32),
        }
        for nm in ("alo", "ahi", "b"):
            gi, dl, nr = buckets[nm][c]
            m[nm + "_idx"] = gi
            m[nm + "_dst"] = dl
            m[nm + "_nrm"] = nr
        in_maps.append(m)
    return in_maps


def kernel(**inputs):
    x_h = np.asarray(inputs["x_h"])
    n = x_h.shape[0]
    cfg, buckets = _prepare(inputs, n)
    nc = build_nc(cfg)
    in_maps = make_in_maps(inputs, cfg, buckets)
    res = run_bass_kernel_spmd(nc, in_maps, core_ids=list(range(NCORES)))
    npc = cfg["NPC"]
    out = np.empty((n, C), np.float32)
    for c in range(NCORES):
        out[c * npc:(c + 1) * npc] = res.results[c]["outT"].T
    return out
